# revision 41
# baseline (speedup 1.0000x reference)
"""Self-contained Trainium2 (Bass/Tile) DeformConv2d kernel.

kernel(x, offset, weight) -> np.ndarray [B, Cout, H, W] float32.
Data-parallel over batch: one SPMD Bass program per NeuronCore (8 cores).

Design (v3):
- All bilinear weights / gather indices / gather table are built on the host
  (numpy) and passed as DRAM inputs; no on-device prep phases.
- Gather table is a float8 e3m4 "pair-row" layout: entry (y, x) holds image
  rows y and y+1 at column x concatenated (2C values). One 4C-value SWDGE
  descriptor per sample covers all 4 bilinear neighbors. e3m4 (range 16,
  4 mantissa bits) halves gather DMA vs bf16 at ~1.3% output rel err
  (e4m3 would be ~2.5% and fail the 2% gate).
- Bilinear weights enter as 128x128 diagonal matrices (ident * w built by
  DVE tensor_scalar, 4x mode, dependent only on host weights so they run
  arbitrarily far ahead) and are fused into the PE transpose: one matmul
  per (piece, jt, cb) multiplies, transposes to channel-major and
  accumulates the 4 taps' pieces in PSUM.
- Per-tap GEMM accumulates out[o, j] over (tap, cb) in PSUM; val and out
  PSUM tiles are split per (cb, column-half) so the Activation-engine
  PSUM->SBUF copies pipeline under the PE stream.
- Tap-0 weight slices load via small SP DMAs, the bulk via the Pool queue,
  so the first gathers are not stuck behind input loads; first/last taps
  are half-gathers to shorten pipeline fill/drain.
- Output is written bf16 and upcast on the host.
Cost-model timeline: ~205.6us/core; PE-bound (91% busy: 123us weighted
transposes + 61us GEMM), gathers 105us on the serialized DMA device.
"""
import sys
import numpy as np
import ml_dtypes

for _p in ("/opt/trn_rl_repo",):
    if _p not in sys.path:
        sys.path.insert(0, _p)

import concourse.bass as bass
import concourse.mybir as mybir
import concourse.tile as tile
from concourse import bacc
from concourse.masks import make_identity
from concourse.bass_utils import run_bass_kernel_spmd

f32 = mybir.dt.float32
bf16 = mybir.dt.bfloat16
fp8 = mybir.dt.float8e3
i16 = mybir.dt.int16
Alu = mybir.AluOpType
P = 128
BF16 = ml_dtypes.bfloat16
FP8 = ml_dtypes.float8_e3m4
USE_FP8 = True


def build_dcn(C=256, Cout=256, H=64, W=64, KH=3, KW=3, CHUNK_JT=8,
              use_fp8=USE_FP8):
    HW = H * W
    S = HW // P              # 32 pixel slots of 128
    NT = KH * KW             # 9 taps
    CB = C // P              # 2 input-channel blocks
    MB = Cout // P           # 2 output-channel blocks
    assert S % CHUNK_JT == 0
    n_chunks = S // CHUNK_JT
    JC = CHUNK_JT * P        # 1024 pixels per chunk
    SWC = JC // 16           # idx columns per chunk (16-wrap layout)
    NNB = JC // 512          # moving-dim blocks for the GEMM
    TROWS = (H + 1) * W      # pair-table rows

    nc = bacc.Bacc("TRN2", target_bir_lowering=False, debug=False)

    # one extra zero row backs the overlapping pair view's last entry
    tdt = fp8 if use_fp8 else bf16
    tbl = nc.declare_dram_parameter("tbl", [TROWS + 1, 2 * C], tdt,
                                    isOutput=False)
    idx = nc.declare_dram_parameter("idx", [P, NT, n_chunks * SWC], i16,
                                    isOutput=False)
    w4 = nc.declare_dram_parameter("w4", [P, NT, S, 4], bf16, isOutput=False)
    wt = nc.declare_dram_parameter("wt", [P, NT, CB, Cout], bf16, isOutput=False)
    out = nc.declare_dram_parameter("out", [Cout, HW], bf16, isOutput=True)

    with tile.TileContext(nc) as tc:
        with tc.tile_pool(name="persist", bufs=1) as pp:
            # tap-0 slices arrive via small SP DMAs so the first gather isn't
            # stuck behind bulk input loads; the k>=1 remainders are loaded on
            # the Pool queue, program-ordered between the first gathers.
            wtb0 = pp.tile([P, 1, CB, Cout], bf16, name="wtb0")
            wtbr = pp.tile([P, NT - 1, CB, Cout], bf16, name="wtbr")
            w4b0 = pp.tile([P, 1, S, 4], bf16, name="w4b0")
            w4br = pp.tile([P, NT - 1, S, 4], bf16, name="w4br")
            w4t0 = pp.tile([P, 1, S, 4], f32, name="w4t0")
            w4tr = pp.tile([P, NT - 1, S, 4], f32, name="w4tr")
            idxt = pp.tile([P, NT, n_chunks * SWC], i16, name="idxt")
            ident = pp.tile([P, P], bf16, name="ident")

            nc.sync.dma_start(out=idxt[:], in_=idx[:])
            nc.sync.dma_start(out=w4b0[:], in_=w4[:, 0:1])
            nc.sync.dma_start(out=wtb0[:], in_=wt[:, 0:1])
            # upconvert weights on DVE (idle at startup)
            nc.vector.tensor_copy(out=w4t0[:], in_=w4b0[:])
            make_identity(nc, ident[:])

            def w4v(k, s_idx, n):
                if k == 0:
                    return w4t0[:, 0, s_idx, n:n + 1]
                return w4tr[:, k - 1, s_idx, n:n + 1]

            def wtv(k, cb, mb):
                if k == 0:
                    return wtb0[:, 0, cb, mb * P:(mb + 1) * P]
                return wtbr[:, k - 1, cb, mb * P:(mb + 1) * P]

            # overlapping-pair view: entry i covers table elements
            # [2C*i, 2C*i + 4C) -> one descriptor = 4 bilinear neighbors.
            tbl_pairs = bass.AP(tbl[:].tensor, 0, [[2 * C, TROWS], [1, 4 * C]])

            HJT = CHUNK_JT // 2          # jts per output-column half
            with (
                tc.tile_pool(name="gather", bufs=4) as g_pool,
                tc.tile_pool(name="prod", bufs=24) as pr_pool,
                tc.tile_pool(name="vout", bufs=8) as v_pool,
                tc.tile_pool(name="obuf", bufs=2) as o_pool,
                tc.tile_pool(name="psum_out", bufs=1, space="PSUM") as pso_pool,
                tc.tile_pool(name="psum_val", bufs=4, space="PSUM") as psv_pool,
            ):
                def do_block(out_ps, ch, k, jt0, njt):
                    """One tap's gather+compute for jts [jt0, jt0+njt)."""
                    ni = njt * P
                    g = g_pool.tile([P, njt, 4 * C], tdt, name="g")
                    col0 = ch * SWC + (jt0 * P) // 16
                    nc.gpsimd.dma_gather(
                        g[:], tbl_pairs,
                        idxt[:, k, col0:col0 + ni // 16],
                        ni, ni, 4 * C, elem_step=2 * C,
                    )
                    # per-jt weight tiles: with fp8 gathers the bilinear
                    # weights enter via diagonal matrices multiplied on the
                    # PE (fused into the transpose); with bf16 they are
                    # applied by DVE tensor_scalar into pr tiles.
                    prs = {}
                    for i in range(njt):
                        jt = jt0 + i
                        s_idx = ch * CHUNK_JT + jt
                        if use_fp8:
                            dg = pr_pool.tile([P, 4, P], bf16, name="dg")
                            prs[jt] = dg
                            for n in range(4):
                                nc.vector.tensor_scalar(
                                    out=dg[:, n, :], in0=ident[:],
                                    scalar1=w4v(k, s_idx, n),
                                    scalar2=None, op0=Alu.mult,
                                )
                        else:
                            pr = pr_pool.tile([P, 4, C], bf16, name="pr")
                            prs[jt] = pr
                            for n in range(4):
                                nc.vector.tensor_scalar(
                                    out=pr[:, n, :],
                                    in0=g[:, i, n * C:(n + 1) * C],
                                    scalar1=w4v(k, s_idx, n),
                                    scalar2=None, op0=Alu.mult,
                                )
                    # val/vsb split per (cb, jt-group of HJT): Act sub-copies
                    # (570ns) start mid-iteration, so no GEMM ever waits
                    # on a copy still in flight
                    vhalf = {}
                    for cb in range(CB):
                        for h0 in range(jt0, jt0 + njt, HJT):
                            nh = min(HJT, jt0 + njt - h0)
                            val_ps = psv_pool.tile([P, nh * P], f32,
                                                   space="PSUM", name="val_ps")
                            for j2 in range(nh):
                                jt = h0 + j2
                                i = jt - jt0
                                for n in range(4):
                                    if use_fp8:
                                        lhsT = g[:, i,
                                                 n * C + cb * P:
                                                 n * C + (cb + 1) * P]
                                        rhs = prs[jt][:, n, :]
                                    else:
                                        lhsT = prs[jt][:, n, cb * P:(cb + 1) * P]
                                        rhs = ident[:]
                                    nc.tensor.matmul(
                                        out=val_ps[:, j2 * P:(j2 + 1) * P],
                                        lhsT=lhsT, rhs=rhs,
                                        start=(n == 0), stop=(n == 3),
                                    )
                            vs = v_pool.tile([P, nh * P], bf16, name="vs")
                            nc.scalar.copy(out=vs[:], in_=val_ps[:])
                            vhalf[(cb, h0)] = (vs, nh)
                    for h0 in range(jt0, jt0 + njt, HJT):
                        for mb in range(MB):
                            for cb in range(CB):
                                vs, nh = vhalf[(cb, h0)]
                                nsl = slice(h0 * P, (h0 + nh) * P)
                                nc.tensor.matmul(
                                    out=out_ps[mb][:, nsl],
                                    lhsT=wtv(k, cb, mb),
                                    rhs=vs[:],
                                    start=(k == 0 and cb == 0),
                                    stop=(k == NT - 1 and cb == CB - 1),
                                )

                for ch in range(n_chunks):
                    out_ps = [
                        pso_pool.tile([P, JC], f32, space="PSUM", name=f"out_ps{m}")
                        for m in range(MB)
                    ]
                    first = ch == 0
                    last_ch = ch == n_chunks - 1
                    for k in range(NT):
                        if (last_ch and k == NT - 1) or (first and k == 0):
                            # split the first/final tap into half-gathers:
                            # shallower pipeline fill and drain
                            do_block(out_ps, ch, k, 0, HJT)
                            do_block(out_ps, ch, k, HJT, HJT)
                        else:
                            do_block(out_ps, ch, k, 0, CHUNK_JT)
                        if first and k == 0:
                            # bulk weight loads ride the Pool queue here so
                            # they cannot front-run the first gathers on the
                            # shared DMA device
                            nc.gpsimd.dma_start(out=w4br[:], in_=w4[:, 1:])
                            nc.gpsimd.dma_start(out=wtbr[:], in_=wt[:, 1:])
                            nc.vector.tensor_copy(out=w4tr[:], in_=w4br[:])
                    for mb in range(MB):
                        ob = o_pool.tile([P, JC], bf16, name="ob")
                        nc.scalar.copy(out=ob[:], in_=out_ps[mb][:])
                        nc.sync.dma_start(
                            out=out[mb * P:(mb + 1) * P, ch * JC:(ch + 1) * JC],
                            in_=ob[:],
                        )

    nc.compile()
    return nc


def host_prep(x_b, offset_b, weight, H, W, KH, KW, PAD):
    """Per-core input map from one batch slice (numpy, f32)."""
    C = x_b.shape[0]
    Cout = weight.shape[0]
    HW = H * W
    S = HW // P
    NT = KH * KW
    CB = C // P

    # pair-row gather table: entry r=(y0+1)*W+x holds rows (y0, y0+1) at col x
    xt = x_b.reshape(C, H, W).transpose(1, 2, 0).astype(np.float32)  # [H, W, C]
    Z = np.zeros((H + 2, W, C), np.float32)
    Z[1:H + 1] = xt
    T = np.concatenate([Z[0:H + 1], Z[1:H + 2]], axis=-1)  # [(H+1), W, 2C]
    tbl = np.zeros(((H + 1) * W + 1, 2 * C), np.float32)
    tbl[:-1] = T.reshape((H + 1) * W, 2 * C)
    tbl = tbl.astype(FP8 if USE_FP8 else BF16)

    # sample coords per (tap, pixel)
    off = offset_b.reshape(NT, 2, HW).astype(np.float32)
    j = np.arange(HW)
    ks = np.arange(NT)
    by = (j[None, :] // W - PAD + (ks // KW)[:, None]).astype(np.float32)
    bx = (j[None, :] % W - PAD + (ks % KW)[:, None]).astype(np.float32)
    py = by + off[:, 0]
    px = bx + off[:, 1]
    y0 = np.floor(py)
    x0 = np.floor(px)
    ly = (py - y0).astype(np.float32)
    lx = (px - x0).astype(np.float32)
    qy = np.clip(y0, -1, H - 1)
    sx = np.clip(x0, 0, W - 2)
    idx_lin = ((qy + 1) * W + sx).astype(np.int16)  # [NT, HW]

    wy0 = (1.0 - ly) * ((y0 >= 0) & (y0 <= H - 1))
    wyB = ly * ((y0 >= -1) & (y0 <= H - 2))
    vx0 = (x0 >= 0) & (x0 <= W - 1)
    vx1 = (x0 >= -1) & (x0 <= W - 2)
    wxA = (1.0 - lx) * vx0 * (x0 == sx) + lx * vx1 * ((x0 + 1) == sx)
    wxB = (1.0 - lx) * vx0 * (x0 == (sx + 1)) + lx * vx1 * ((x0 + 1) == (sx + 1))
    # piece order matches the gathered 4C row: [y0|x0, y1|x0, y0|x1, y1|x1]
    w4 = np.stack([wy0 * wxA, wyB * wxA, wy0 * wxB, wyB * wxB],
                  axis=-1).astype(np.float32)  # [NT, HW, 4]
    w4d = np.ascontiguousarray(
        w4.reshape(NT, S, P, 4).transpose(2, 0, 1, 3)).astype(BF16)  # [P,NT,S,4]

    # 16-wrap idx layout: slice column c of chunk ch, partition q -> sample
    # i = c*16 + q (i = chunk-local pixel), replicated over 8 partition groups
    idxw = idx_lin.reshape(NT, HW // 16, 16).transpose(2, 0, 1)  # [16, NT, HW/16]
    idxw = np.ascontiguousarray(np.tile(idxw, (8, 1, 1))).astype(np.int16)

    wtd = np.ascontiguousarray(
        weight.reshape(Cout, CB, P, NT).transpose(2, 3, 1, 0)).astype(BF16)
    return {"tbl": tbl, "idx": idxw, "w4": w4d, "wt": wtd}


_NC_CACHE = {}


def _get_nc(key, **kw):
    if key not in _NC_CACHE:
        _NC_CACHE[key] = build_dcn(**kw)
    return _NC_CACHE[key]


def kernel(x, offset, weight):
    x = np.asarray(x, dtype=np.float32)
    offset = np.asarray(offset, dtype=np.float32)
    weight = np.asarray(weight, dtype=np.float32)
    B, C, H, W = x.shape
    Cout = weight.shape[0]
    KH, KW = weight.shape[2], weight.shape[3]
    assert B == 8 and C % 128 == 0 and Cout % 128 == 0
    nc = _get_nc((C, Cout, H, W, KH, KW), C=C, Cout=Cout, H=H, W=W,
                 KH=KH, KW=KW, CHUNK_JT=8)
    in_maps = [host_prep(x[b], offset[b], weight, H, W, KH, KW, 1)
               for b in range(B)]
    res = run_bass_kernel_spmd(nc, in_maps, list(range(B)))
    out = np.stack([
        np.asarray(res.results[b]["out"]).astype(np.float32).reshape(Cout, H, W)
        for b in range(B)
    ])
    return out


# revision 42
# speedup vs baseline: 1.2036x; 1.2036x over previous
"""Self-contained Trainium2 (Bass/Tile) DeformConv2d kernel.

kernel(x, offset, weight) -> np.ndarray [B, Cout, H, W] float32.
Data-parallel over batch: one SPMD Bass program per NeuronCore (8 cores).

Design (v4):
- All bilinear weights / gather indices / gather table are built on the host
  (numpy) and passed as DRAM inputs; no on-device prep phases.
- Gather table is a float8 e3m4 "pair-row" layout: entry (y, x) holds image
  rows y and y+1 at column x concatenated (2C values, 512B). Each sample
  issues TWO 512B descriptors (columns x0, x0+1) landing on adjacent SBUF
  partitions: 64 samples x 2 x-pieces per 128-partition group. e3m4 halves
  gather DMA vs bf16 at ~1.3% output rel err (e4m3 would be ~2.5%, failing
  the 2% gate).
- Bilinear weights enter as 128x64 two-diagonal matrices (I2 mask * per-
  partition weight, built by DVE tensor_scalar in 4x mode; they depend only
  on host data so they run ahead). One PE matmul per (y-half, group, cb)
  multiplies, transposes to channel-major AND reduces both x-pieces:
  out[c, j] = w_x0*g_x0[c, j] + w_x1*g_x1[c, j]; the two y-halves accumulate
  in PSUM. This is 2x fewer PE transpose cycles than one-piece-per-matmul.
- Per-tap GEMM accumulates out[o, j] over (tap, cb) in PSUM; val and out
  PSUM tiles are split per (cb, column-half) so the Activation-engine
  PSUM->SBUF copies pipeline under the PE stream.
- Tap-0 weight slices load via small SP DMAs, the bulk via the Pool queue,
  so the first gathers are not stuck behind input loads; first/last taps
  are half-gathers to shorten pipeline fill/drain.
- Output is written bf16 and upcast on the host.
"""
import sys
import numpy as np
import ml_dtypes

for _p in ("/opt/trn_rl_repo",):
    if _p not in sys.path:
        sys.path.insert(0, _p)

import concourse.bass as bass
import concourse.mybir as mybir
import concourse.tile as tile
from concourse import bacc
from concourse.bass_utils import run_bass_kernel_spmd

f32 = mybir.dt.float32
bf16 = mybir.dt.bfloat16
fp8 = mybir.dt.float8e3
i16 = mybir.dt.int16
Alu = mybir.AluOpType
P = 128
BF16 = ml_dtypes.bfloat16
FP8 = ml_dtypes.float8_e3m4


def build_dcn(C=256, Cout=256, H=64, W=64, KH=3, KW=3, CHUNK_JT=8):
    HW = H * W
    S = HW // P              # 32 pixel slots of 128
    NT = KH * KW             # 9 taps
    CB = C // P              # 2 input-channel blocks
    MB = Cout // P           # 2 output-channel blocks
    assert S % CHUNK_JT == 0
    n_chunks = S // CHUNK_JT
    JC = CHUNK_JT * P        # 1024 pixels per chunk
    NG = HW // 64            # 64-sample groups over the image
    GC = JC // 64            # groups per chunk (16)
    TROWS = (H + 1) * W      # pair-table rows

    nc = bacc.Bacc("TRN2", target_bir_lowering=False, debug=False)

    tbl = nc.declare_dram_parameter("tbl", [TROWS, 2 * C], fp8, isOutput=False)
    # two idx entries per sample (x0, x0+1), 16-wrap layout
    idx = nc.declare_dram_parameter("idx", [P, NT, 2 * HW // 16], i16,
                                    isOutput=False)
    # per-partition 2-diagonal weights: [p, k, group, y-half]
    w2 = nc.declare_dram_parameter("w2", [P, NT, NG, 2], bf16, isOutput=False)
    wt = nc.declare_dram_parameter("wt", [P, NT, CB, Cout], bf16, isOutput=False)
    i2m = nc.declare_dram_parameter("i2m", [P, 64], bf16, isOutput=False)
    out = nc.declare_dram_parameter("out", [Cout, HW], bf16, isOutput=True)

    with tile.TileContext(nc) as tc:
        with tc.tile_pool(name="persist", bufs=1) as pp:
            # tap-0 slices arrive via small SP DMAs so the first gather isn't
            # stuck behind bulk input loads; the k>=1 remainders are loaded on
            # the Pool queue, program-ordered between the first gathers.
            wtb0 = pp.tile([P, 1, CB, Cout], bf16, name="wtb0")
            wtbr = pp.tile([P, NT - 1, CB, Cout], bf16, name="wtbr")
            w2b0 = pp.tile([P, 1, NG, 2], bf16, name="w2b0")
            w2br = pp.tile([P, NT - 1, NG, 2], bf16, name="w2br")
            w2t0 = pp.tile([P, 1, NG, 2], f32, name="w2t0")
            w2tr = pp.tile([P, NT - 1, NG, 2], f32, name="w2tr")
            idxt = pp.tile([P, NT, 2 * HW // 16], i16, name="idxt")
            i2t = pp.tile([P, 64], bf16, name="i2t")

            nc.sync.dma_start(out=i2t[:], in_=i2m[:])
            nc.sync.dma_start(out=idxt[:], in_=idx[:])
            nc.sync.dma_start(out=w2b0[:], in_=w2[:, 0:1])
            nc.sync.dma_start(out=wtb0[:], in_=wt[:, 0:1])
            # upconvert weights on DVE (idle at startup)
            nc.vector.tensor_copy(out=w2t0[:], in_=w2b0[:])

            def w2v(k, g, yh):
                if k == 0:
                    return w2t0[:, 0, g, yh:yh + 1]
                return w2tr[:, k - 1, g, yh:yh + 1]

            def wtv(k, cb, mb):
                if k == 0:
                    return wtb0[:, 0, cb, mb * P:(mb + 1) * P]
                return wtbr[:, k - 1, cb, mb * P:(mb + 1) * P]

            tbl_rows = bass.AP(tbl[:].tensor, 0, [[2 * C, TROWS], [1, 2 * C]])

            HGC = GC // 2            # groups per output-column half (8)
            with (
                tc.tile_pool(name="gather", bufs=4) as g_pool,
                tc.tile_pool(name="prod", bufs=96) as pr_pool,
                tc.tile_pool(name="vout", bufs=8) as v_pool,
                tc.tile_pool(name="obuf", bufs=2) as o_pool,
                tc.tile_pool(name="psum_out", bufs=1, space="PSUM") as pso_pool,
                tc.tile_pool(name="psum_val", bufs=4, space="PSUM") as psv_pool,
            ):
                def do_block(out_ps, ch, k, g0, ng):
                    """One tap's gather+compute for groups [g0, g0+ng)."""
                    nrows = ng * P           # 2 rows per sample, 64/group
                    g = g_pool.tile([P, ng, 2 * C], fp8, name="g")
                    col0 = (ch * GC + g0) * 8
                    nc.gpsimd.dma_gather(
                        g[:], tbl_rows,
                        idxt[:, k, col0:col0 + ng * 8],
                        nrows, nrows, 2 * C, elem_step=2 * C,
                    )
                    # per-group 2-diagonal weight tiles (I2 * w), DVE 4x mode
                    dgs = {}
                    for i in range(ng):
                        gg = g0 + i
                        g_glob = ch * GC + gg
                        dg = pr_pool.tile([P, 2, 64], bf16, name="dg")
                        dgs[gg] = dg
                        for yh in range(2):
                            nc.vector.tensor_scalar(
                                out=dg[:, yh, :], in0=i2t[:],
                                scalar1=w2v(k, g_glob, yh),
                                scalar2=None, op0=Alu.mult,
                            )
                    # val split per (cb, column-half): Act sub-copies start
                    # mid-iteration, so no GEMM waits on a copy in flight
                    vhalf = {}
                    for cb in range(CB):
                        for h0 in range(g0, g0 + ng, HGC):
                            nh = min(HGC, g0 + ng - h0)
                            val_ps = psv_pool.tile([P, nh * 64], f32,
                                                   space="PSUM", name="val_ps")
                            for j2 in range(nh):
                                gg = h0 + j2
                                i = gg - g0
                                for yh in range(2):
                                    nc.tensor.matmul(
                                        out=val_ps[:, j2 * 64:(j2 + 1) * 64],
                                        lhsT=g[:, i,
                                               yh * C + cb * P:
                                               yh * C + (cb + 1) * P],
                                        rhs=dgs[gg][:, yh, :],
                                        start=(yh == 0), stop=(yh == 1),
                                    )
                            vs = v_pool.tile([P, nh * 64], bf16, name="vs")
                            nc.scalar.copy(out=vs[:], in_=val_ps[:])
                            vhalf[(cb, h0)] = (vs, nh)
                    for h0 in range(g0, g0 + ng, HGC):
                        for mb in range(MB):
                            for cb in range(CB):
                                vs, nh = vhalf[(cb, h0)]
                                nsl = slice(h0 * 64, (h0 + nh) * 64)
                                nc.tensor.matmul(
                                    out=out_ps[mb][:, nsl],
                                    lhsT=wtv(k, cb, mb),
                                    rhs=vs[:],
                                    start=(k == 0 and cb == 0),
                                    stop=(k == NT - 1 and cb == CB - 1),
                                )

                for ch in range(n_chunks):
                    out_ps = [
                        pso_pool.tile([P, JC], f32, space="PSUM", name=f"out_ps{m}")
                        for m in range(MB)
                    ]
                    first = ch == 0
                    last_ch = ch == n_chunks - 1
                    for k in range(NT):
                        if (last_ch and k == NT - 1) or (first and k == 0):
                            # split the first/final tap into half-gathers:
                            # shallower pipeline fill and drain
                            do_block(out_ps, ch, k, 0, HGC)
                            do_block(out_ps, ch, k, HGC, HGC)
                        else:
                            do_block(out_ps, ch, k, 0, GC)
                        if first and k == 0:
                            # bulk weight loads ride the Pool queue here so
                            # they cannot front-run the first gathers on the
                            # shared DMA device
                            nc.gpsimd.dma_start(out=w2br[:], in_=w2[:, 1:])
                            nc.gpsimd.dma_start(out=wtbr[:], in_=wt[:, 1:])
                            nc.vector.tensor_copy(out=w2tr[:], in_=w2br[:])
                    for mb in range(MB):
                        ob = o_pool.tile([P, JC], bf16, name="ob")
                        nc.scalar.copy(out=ob[:], in_=out_ps[mb][:])
                        nc.sync.dma_start(
                            out=out[mb * P:(mb + 1) * P, ch * JC:(ch + 1) * JC],
                            in_=ob[:],
                        )

    nc.compile()
    return nc


def host_prep(x_b, offset_b, weight, H, W, KH, KW, PAD):
    """Per-core input map from one batch slice (numpy, f32)."""
    C = x_b.shape[0]
    Cout = weight.shape[0]
    HW = H * W
    NT = KH * KW
    CB = C // P
    NG = HW // 64

    # pair-row gather table: entry r=(y0+1)*W+x holds rows (y0, y0+1) at col x
    xt = x_b.reshape(C, H, W).transpose(1, 2, 0).astype(np.float32)  # [H, W, C]
    Z = np.zeros((H + 2, W, C), np.float32)
    Z[1:H + 1] = xt
    T = np.concatenate([Z[0:H + 1], Z[1:H + 2]], axis=-1)  # [(H+1), W, 2C]
    tbl = T.reshape((H + 1) * W, 2 * C).astype(FP8)

    # sample coords per (tap, pixel)
    off = offset_b.reshape(NT, 2, HW).astype(np.float32)
    j = np.arange(HW)
    ks = np.arange(NT)
    by = (j[None, :] // W - PAD + (ks // KW)[:, None]).astype(np.float32)
    bx = (j[None, :] % W - PAD + (ks % KW)[:, None]).astype(np.float32)
    py = by + off[:, 0]
    px = bx + off[:, 1]
    y0 = np.floor(py)
    x0 = np.floor(px)
    ly = (py - y0).astype(np.float32)
    lx = (px - x0).astype(np.float32)
    qy = np.clip(y0, -1, H - 1)
    sx = np.clip(x0, 0, W - 2)
    base = ((qy + 1) * W + sx).astype(np.int32)  # [NT, HW]

    wy0 = (1.0 - ly) * ((y0 >= 0) & (y0 <= H - 1))
    wyB = ly * ((y0 >= -1) & (y0 <= H - 2))
    vx0 = (x0 >= 0) & (x0 <= W - 1)
    vx1 = (x0 >= -1) & (x0 <= W - 2)
    wxA = (1.0 - lx) * vx0 * (x0 == sx) + lx * vx1 * ((x0 + 1) == sx)
    wxB = (1.0 - lx) * vx0 * (x0 == (sx + 1)) + lx * vx1 * ((x0 + 1) == (sx + 1))
    # piece order: [y0|x0, y1|x0, y0|x1, y1|x1]
    w4 = np.stack([wy0 * wxA, wyB * wxA, wy0 * wxB, wyB * wxB],
                  axis=-1).astype(np.float32)  # [NT, HW, 4]

    # two-diagonal weights: partition p of group g holds sample 64g + p//2,
    # x-piece p%2; y-half yh selects row y0/y1 -> piece index yh + 2*(p%2)
    w4r = w4.reshape(NT, NG, 64, 4)
    pp_ = np.arange(P)
    pc = pp_ // 2
    pxp = pp_ % 2
    w2d = np.empty((P, NT, NG, 2), np.float32)
    for yh in range(2):
        piece = yh + 2 * pxp                        # [128]
        sel = w4r[:, :, pc, :]                      # [NT, NG, 128, 4]
        w2d[:, :, :, yh] = sel[:, :, np.arange(P), piece].transpose(2, 0, 1)
    w2d = np.ascontiguousarray(w2d).astype(BF16)

    # interleaved idx rows: sample i -> entries (base, base+1), 16-wrap
    idx2 = np.empty((NT, 2 * HW), np.int32)
    idx2[:, 0::2] = base
    idx2[:, 1::2] = base + 1
    idxw = idx2.reshape(NT, (2 * HW) // 16, 16).transpose(2, 0, 1)
    idxw = np.ascontiguousarray(np.tile(idxw, (8, 1, 1))).astype(np.int16)

    i2m = np.zeros((P, 64), np.float32)
    i2m[pp_, pc] = 1.0
    i2m = i2m.astype(BF16)

    wtd = np.ascontiguousarray(
        weight.reshape(Cout, CB, P, NT).transpose(2, 3, 1, 0)).astype(BF16)
    return {"tbl": tbl, "idx": idxw, "w2": w2d, "wt": wtd, "i2m": i2m}


_NC_CACHE = {}


def _get_nc(key, **kw):
    if key not in _NC_CACHE:
        _NC_CACHE[key] = build_dcn(**kw)
    return _NC_CACHE[key]


def kernel(x, offset, weight):
    x = np.asarray(x, dtype=np.float32)
    offset = np.asarray(offset, dtype=np.float32)
    weight = np.asarray(weight, dtype=np.float32)
    B, C, H, W = x.shape
    Cout = weight.shape[0]
    KH, KW = weight.shape[2], weight.shape[3]
    assert B == 8 and C % 128 == 0 and Cout % 128 == 0
    nc = _get_nc((C, Cout, H, W, KH, KW), C=C, Cout=Cout, H=H, W=W,
                 KH=KH, KW=KW, CHUNK_JT=8)
    in_maps = [host_prep(x[b], offset[b], weight, H, W, KH, KW, 1)
               for b in range(B)]
    res = run_bass_kernel_spmd(nc, in_maps, list(range(B)))
    out = np.stack([
        np.asarray(res.results[b]["out"]).astype(np.float32).reshape(Cout, H, W)
        for b in range(B)
    ])
    return out


# revision 43
# speedup vs baseline: 1.2601x; 1.0469x over previous
"""Self-contained Trainium2 (Bass/Tile) DeformConv2d kernel.

kernel(x, offset, weight) -> np.ndarray [B, Cout, H, W] float32.
Data-parallel over batch: one SPMD Bass program per NeuronCore (8 cores).

Design (v4):
- All bilinear weights / gather indices / gather table are built on the host
  (numpy) and passed as DRAM inputs; no on-device prep phases.
- Gather table is a float8 e3m4 "pair-row" layout: entry (y, x) holds image
  rows y and y+1 at column x concatenated (2C values, 512B). Each sample
  issues TWO 512B descriptors (columns x0, x0+1) landing on adjacent SBUF
  partitions: 64 samples x 2 x-pieces per 128-partition group. e3m4 halves
  gather DMA vs bf16 at ~1.3% output rel err (e4m3 would be ~2.5%, failing
  the 2% gate).
- Bilinear weights enter as 128x64 two-diagonal matrices (I2 mask * per-
  partition weight, built by DVE tensor_scalar in 4x mode; they depend only
  on host data so they run ahead). One PE matmul per (y-half, group, cb)
  multiplies, transposes to channel-major AND reduces both x-pieces:
  out[c, j] = w_x0*g_x0[c, j] + w_x1*g_x1[c, j]; the two y-halves accumulate
  in PSUM. This is 2x fewer PE transpose cycles than one-piece-per-matmul.
- Per-tap GEMM accumulates out[o, j] over (tap, cb) in PSUM; val and out
  PSUM tiles are split per (cb, column-half) so the Activation-engine
  PSUM->SBUF copies pipeline under the PE stream.
- Tap-0 weight slices load via small SP DMAs, the bulk via the Pool queue,
  so the first gathers are not stuck behind input loads; first/last taps
  are half-gathers to shorten pipeline fill/drain.
- Output is written bf16 and upcast on the host.
"""
import sys
import numpy as np
import ml_dtypes

for _p in ("/opt/trn_rl_repo",):
    if _p not in sys.path:
        sys.path.insert(0, _p)

import concourse.bass as bass
import concourse.mybir as mybir
import concourse.tile as tile
from concourse import bacc
from concourse.bass_utils import run_bass_kernel_spmd

f32 = mybir.dt.float32
bf16 = mybir.dt.bfloat16
fp8 = mybir.dt.float8e3
i16 = mybir.dt.int16
Alu = mybir.AluOpType
P = 128
BF16 = ml_dtypes.bfloat16
FP8 = ml_dtypes.float8_e3m4


def build_dcn(C=256, Cout=256, H=64, W=64, KH=3, KW=3, CHUNK_JT=8):
    HW = H * W
    S = HW // P              # 32 pixel slots of 128
    NT = KH * KW             # 9 taps
    CB = C // P              # 2 input-channel blocks
    MB = Cout // P           # 2 output-channel blocks
    assert S % CHUNK_JT == 0
    n_chunks = S // CHUNK_JT
    JC = CHUNK_JT * P        # 1024 pixels per chunk
    NG = HW // 64            # 64-sample groups over the image
    GC = JC // 64            # groups per chunk (16)
    TROWS = (H + 1) * W      # pair-table rows

    nc = bacc.Bacc("TRN2", target_bir_lowering=False, debug=False)

    tbl = nc.declare_dram_parameter("tbl", [TROWS, 2 * C], fp8, isOutput=False)
    # two idx entries per sample (x0, x0+1), 16-wrap layout
    idx = nc.declare_dram_parameter("idx", [P, NT, 2 * HW // 16], i16,
                                    isOutput=False)
    # per-partition 2-diagonal weights: [p, k, group, y-half]
    w2 = nc.declare_dram_parameter("w2", [P, NT, NG, 2], bf16, isOutput=False)
    wt = nc.declare_dram_parameter("wt", [P, NT, CB, Cout], bf16, isOutput=False)
    i2m = nc.declare_dram_parameter("i2m", [P, 64], bf16, isOutput=False)
    out = nc.declare_dram_parameter("out", [Cout, HW], bf16, isOutput=True)

    with tile.TileContext(nc) as tc:
        with tc.tile_pool(name="persist", bufs=1) as pp:
            # tap-0 slices arrive via small SP DMAs so the first gather isn't
            # stuck behind bulk input loads; the k>=1 remainders are loaded on
            # the Pool queue, program-ordered between the first gathers.
            wtb0 = pp.tile([P, 1, CB, Cout], bf16, name="wtb0")
            wtbr = pp.tile([P, NT - 1, CB, Cout], bf16, name="wtbr")
            w2b0 = pp.tile([P, 1, NG, 2], bf16, name="w2b0")
            w2br = pp.tile([P, NT - 1, NG, 2], bf16, name="w2br")
            w2t0 = pp.tile([P, 1, NG, 2], f32, name="w2t0")
            w2tr = pp.tile([P, NT - 1, NG, 2], f32, name="w2tr")
            idxt = pp.tile([P, NT, 2 * HW // 16], i16, name="idxt")
            i2t = pp.tile([P, 64], bf16, name="i2t")

            nc.sync.dma_start(out=i2t[:], in_=i2m[:])
            nc.sync.dma_start(out=idxt[:], in_=idx[:])
            nc.sync.dma_start(out=w2b0[:], in_=w2[:, 0:1])
            nc.sync.dma_start(out=wtb0[:], in_=wt[:, 0:1])
            # upconvert weights on DVE (idle at startup)
            nc.vector.tensor_copy(out=w2t0[:], in_=w2b0[:])

            def w2v(k, g, yh):
                if k == 0:
                    return w2t0[:, 0, g, yh:yh + 1]
                return w2tr[:, k - 1, g, yh:yh + 1]

            def wtv(k, cb, mb):
                if k == 0:
                    return wtb0[:, 0, cb, mb * P:(mb + 1) * P]
                return wtbr[:, k - 1, cb, mb * P:(mb + 1) * P]

            tbl_rows = bass.AP(tbl[:].tensor, 0, [[2 * C, TROWS], [1, 2 * C]])

            HGC = GC // 2            # groups per output-column half (8)
            with (
                tc.tile_pool(name="gather", bufs=4) as g_pool,
                tc.tile_pool(name="prod", bufs=96) as pr_pool,
                tc.tile_pool(name="vout", bufs=8) as v_pool,
                tc.tile_pool(name="obuf", bufs=2) as o_pool,
                tc.tile_pool(name="psum_out", bufs=1, space="PSUM") as pso_pool,
                tc.tile_pool(name="psum_val", bufs=4, space="PSUM") as psv_pool,
            ):
                def do_block(out_ps, ch, k, g0, ng):
                    """One tap's gather+compute for groups [g0, g0+ng)."""
                    nrows = ng * P           # 2 rows per sample, 64/group
                    g = g_pool.tile([P, ng, 2 * C], fp8, name="g")
                    col0 = (ch * GC + g0) * 8
                    nc.gpsimd.dma_gather(
                        g[:], tbl_rows,
                        idxt[:, k, col0:col0 + ng * 8],
                        nrows, nrows, 2 * C, elem_step=2 * C,
                    )
                    # per-group 2-diagonal weight tiles (I2 * w), DVE 4x mode
                    dgs = {}
                    for i in range(ng):
                        gg = g0 + i
                        g_glob = ch * GC + gg
                        dg = pr_pool.tile([P, 2, 64], bf16, name="dg")
                        dgs[gg] = dg
                        for yh in range(2):
                            nc.vector.tensor_scalar(
                                out=dg[:, yh, :], in0=i2t[:],
                                scalar1=w2v(k, g_glob, yh),
                                scalar2=None, op0=Alu.mult,
                            )
                    # val split per (cb, column-half): Act sub-copies start
                    # mid-iteration, so no GEMM waits on a copy in flight
                    vhalf = {}
                    for cb in range(CB):
                        for h0 in range(g0, g0 + ng, HGC):
                            nh = min(HGC, g0 + ng - h0)
                            val_ps = psv_pool.tile([P, nh * 64], f32,
                                                   space="PSUM", name="val_ps")
                            for j2 in range(nh):
                                gg = h0 + j2
                                i = gg - g0
                                for yh in range(2):
                                    nc.tensor.matmul(
                                        out=val_ps[:, j2 * 64:(j2 + 1) * 64],
                                        lhsT=g[:, i,
                                               yh * C + cb * P:
                                               yh * C + (cb + 1) * P],
                                        rhs=dgs[gg][:, yh, :],
                                        start=(yh == 0), stop=(yh == 1),
                                    )
                            vs = v_pool.tile([P, nh * 64], bf16, name="vs")
                            nc.scalar.copy(out=vs[:], in_=val_ps[:])
                            vhalf[(cb, h0)] = (vs, nh)
                    # cb-major GEMM order: each GEMM's vs copy is the one
                    # that completed earliest on the Act queue -> no PE waits
                    for cb in range(CB):
                        for h0 in range(g0, g0 + ng, HGC):
                            for mb in range(MB):
                                vs, nh = vhalf[(cb, h0)]
                                nsl = slice(h0 * 64, (h0 + nh) * 64)
                                nc.tensor.matmul(
                                    out=out_ps[mb][:, nsl],
                                    lhsT=wtv(k, cb, mb),
                                    rhs=vs[:],
                                    start=(k == 0 and cb == 0),
                                    stop=(k == NT - 1 and cb == CB - 1),
                                )

                for ch in range(n_chunks):
                    out_ps = [
                        pso_pool.tile([P, JC], f32, space="PSUM", name=f"out_ps{m}")
                        for m in range(MB)
                    ]
                    first = ch == 0
                    last_ch = ch == n_chunks - 1
                    for k in range(NT):
                        if (last_ch and k == NT - 1) or (first and k == 0):
                            # split the first/final tap into half-gathers:
                            # shallower pipeline fill and drain
                            do_block(out_ps, ch, k, 0, HGC)
                            do_block(out_ps, ch, k, HGC, HGC)
                        else:
                            do_block(out_ps, ch, k, 0, GC)
                        if first and k == 0:
                            # bulk weight loads ride the Pool queue here so
                            # they cannot front-run the first gathers on the
                            # shared DMA device
                            nc.gpsimd.dma_start(out=w2br[:], in_=w2[:, 1:])
                            nc.gpsimd.dma_start(out=wtbr[:], in_=wt[:, 1:])
                            nc.vector.tensor_copy(out=w2tr[:], in_=w2br[:])
                    for mb in range(MB):
                        ob = o_pool.tile([P, JC], bf16, name="ob")
                        nc.scalar.copy(out=ob[:], in_=out_ps[mb][:])
                        nc.sync.dma_start(
                            out=out[mb * P:(mb + 1) * P, ch * JC:(ch + 1) * JC],
                            in_=ob[:],
                        )

    nc.compile()
    return nc


def host_prep(x_b, offset_b, weight, H, W, KH, KW, PAD):
    """Per-core input map from one batch slice (numpy, f32)."""
    C = x_b.shape[0]
    Cout = weight.shape[0]
    HW = H * W
    NT = KH * KW
    CB = C // P
    NG = HW // 64

    # pair-row gather table: entry r=(y0+1)*W+x holds rows (y0, y0+1) at col x
    xt = x_b.reshape(C, H, W).transpose(1, 2, 0).astype(np.float32)  # [H, W, C]
    Z = np.zeros((H + 2, W, C), np.float32)
    Z[1:H + 1] = xt
    T = np.concatenate([Z[0:H + 1], Z[1:H + 2]], axis=-1)  # [(H+1), W, 2C]
    tbl = T.reshape((H + 1) * W, 2 * C).astype(FP8)

    # sample coords per (tap, pixel)
    off = offset_b.reshape(NT, 2, HW).astype(np.float32)
    j = np.arange(HW)
    ks = np.arange(NT)
    by = (j[None, :] // W - PAD + (ks // KW)[:, None]).astype(np.float32)
    bx = (j[None, :] % W - PAD + (ks % KW)[:, None]).astype(np.float32)
    py = by + off[:, 0]
    px = bx + off[:, 1]
    y0 = np.floor(py)
    x0 = np.floor(px)
    ly = (py - y0).astype(np.float32)
    lx = (px - x0).astype(np.float32)
    qy = np.clip(y0, -1, H - 1)
    sx = np.clip(x0, 0, W - 2)
    base = ((qy + 1) * W + sx).astype(np.int32)  # [NT, HW]

    wy0 = (1.0 - ly) * ((y0 >= 0) & (y0 <= H - 1))
    wyB = ly * ((y0 >= -1) & (y0 <= H - 2))
    vx0 = (x0 >= 0) & (x0 <= W - 1)
    vx1 = (x0 >= -1) & (x0 <= W - 2)
    wxA = (1.0 - lx) * vx0 * (x0 == sx) + lx * vx1 * ((x0 + 1) == sx)
    wxB = (1.0 - lx) * vx0 * (x0 == (sx + 1)) + lx * vx1 * ((x0 + 1) == (sx + 1))
    # piece order: [y0|x0, y1|x0, y0|x1, y1|x1]
    w4 = np.stack([wy0 * wxA, wyB * wxA, wy0 * wxB, wyB * wxB],
                  axis=-1).astype(np.float32)  # [NT, HW, 4]

    # two-diagonal weights: partition p of group g holds sample 64g + p//2,
    # x-piece p%2; y-half yh selects row y0/y1 -> piece index yh + 2*(p%2)
    w4r = w4.reshape(NT, NG, 64, 4)
    pp_ = np.arange(P)
    pc = pp_ // 2
    pxp = pp_ % 2
    w2d = np.empty((P, NT, NG, 2), np.float32)
    for yh in range(2):
        piece = yh + 2 * pxp                        # [128]
        sel = w4r[:, :, pc, :]                      # [NT, NG, 128, 4]
        w2d[:, :, :, yh] = sel[:, :, np.arange(P), piece].transpose(2, 0, 1)
    w2d = np.ascontiguousarray(w2d).astype(BF16)

    # interleaved idx rows: sample i -> entries (base, base+1), 16-wrap
    idx2 = np.empty((NT, 2 * HW), np.int32)
    idx2[:, 0::2] = base
    idx2[:, 1::2] = base + 1
    idxw = idx2.reshape(NT, (2 * HW) // 16, 16).transpose(2, 0, 1)
    idxw = np.ascontiguousarray(np.tile(idxw, (8, 1, 1))).astype(np.int16)

    i2m = np.zeros((P, 64), np.float32)
    i2m[pp_, pc] = 1.0
    i2m = i2m.astype(BF16)

    wtd = np.ascontiguousarray(
        weight.reshape(Cout, CB, P, NT).transpose(2, 3, 1, 0)).astype(BF16)
    return {"tbl": tbl, "idx": idxw, "w2": w2d, "wt": wtd, "i2m": i2m}


_NC_CACHE = {}


def _get_nc(key, **kw):
    if key not in _NC_CACHE:
        _NC_CACHE[key] = build_dcn(**kw)
    return _NC_CACHE[key]


def kernel(x, offset, weight):
    x = np.asarray(x, dtype=np.float32)
    offset = np.asarray(offset, dtype=np.float32)
    weight = np.asarray(weight, dtype=np.float32)
    B, C, H, W = x.shape
    Cout = weight.shape[0]
    KH, KW = weight.shape[2], weight.shape[3]
    assert B == 8 and C % 128 == 0 and Cout % 128 == 0
    nc = _get_nc((C, Cout, H, W, KH, KW), C=C, Cout=Cout, H=H, W=W,
                 KH=KH, KW=KW, CHUNK_JT=8)
    in_maps = [host_prep(x[b], offset[b], weight, H, W, KH, KW, 1)
               for b in range(B)]
    res = run_bass_kernel_spmd(nc, in_maps, list(range(B)))
    out = np.stack([
        np.asarray(res.results[b]["out"]).astype(np.float32).reshape(Cout, H, W)
        for b in range(B)
    ])
    return out


# revision 44
# speedup vs baseline: 1.2653x; 1.0041x over previous
"""Self-contained Trainium2 (Bass/Tile) DeformConv2d kernel.

kernel(x, offset, weight) -> np.ndarray [B, Cout, H, W] float32.
Data-parallel over batch: one SPMD Bass program per NeuronCore (8 cores).

Design (v4):
- All bilinear weights / gather indices / gather table are built on the host
  (numpy) and passed as DRAM inputs; no on-device prep phases.
- Gather table is a float8 e3m4 "pair-row" layout: entry (y, x) holds image
  rows y and y+1 at column x concatenated (2C values, 512B). Each sample
  issues TWO 512B descriptors (columns x0, x0+1) landing on adjacent SBUF
  partitions: 64 samples x 2 x-pieces per 128-partition group. e3m4 halves
  gather DMA vs bf16 at ~1.3% output rel err (e4m3 would be ~2.5%, failing
  the 2% gate).
- Bilinear weights enter as 128x64 two-diagonal matrices (I2 mask * per-
  partition weight, built by DVE tensor_scalar in 4x mode; they depend only
  on host data so they run ahead). One PE matmul per (y-half, group, cb)
  multiplies, transposes to channel-major AND reduces both x-pieces:
  out[c, j] = w_x0*g_x0[c, j] + w_x1*g_x1[c, j]; the two y-halves accumulate
  in PSUM. This is 2x fewer PE transpose cycles than one-piece-per-matmul.
- Per-tap GEMM accumulates out[o, j] over (tap, cb) in PSUM; val and out
  PSUM tiles are split per (cb, column-half) so the Activation-engine
  PSUM->SBUF copies pipeline under the PE stream.
- Tap-0 weight slices load via small SP DMAs, the bulk via the Pool queue,
  so the first gathers are not stuck behind input loads; first/last taps
  are half-gathers to shorten pipeline fill/drain.
- Output is written bf16 and upcast on the host.
"""
import sys
import numpy as np
import ml_dtypes

for _p in ("/opt/trn_rl_repo",):
    if _p not in sys.path:
        sys.path.insert(0, _p)

import concourse.bass as bass
import concourse.mybir as mybir
import concourse.tile as tile
from concourse import bacc
from concourse.bass_utils import run_bass_kernel_spmd

f32 = mybir.dt.float32
bf16 = mybir.dt.bfloat16
fp8 = mybir.dt.float8e3
i16 = mybir.dt.int16
Alu = mybir.AluOpType
P = 128
BF16 = ml_dtypes.bfloat16
FP8 = ml_dtypes.float8_e3m4


def build_dcn(C=256, Cout=256, H=64, W=64, KH=3, KW=3, CHUNK_JT=8):
    HW = H * W
    S = HW // P              # 32 pixel slots of 128
    NT = KH * KW             # 9 taps
    CB = C // P              # 2 input-channel blocks
    MB = Cout // P           # 2 output-channel blocks
    assert S % CHUNK_JT == 0
    n_chunks = S // CHUNK_JT
    JC = CHUNK_JT * P        # 1024 pixels per chunk
    NG = HW // 64            # 64-sample groups over the image
    GC = JC // 64            # groups per chunk (16)
    TROWS = (H + 1) * W      # pair-table rows

    nc = bacc.Bacc("TRN2", target_bir_lowering=False, debug=False)

    tbl = nc.declare_dram_parameter("tbl", [TROWS, 2 * C], fp8, isOutput=False)
    # two idx entries per sample (x0, x0+1), 16-wrap layout
    idx = nc.declare_dram_parameter("idx", [P, NT, 2 * HW // 16], i16,
                                    isOutput=False)
    # per-partition 2-diagonal weights: [p, k, group, y-half]
    w2 = nc.declare_dram_parameter("w2", [P, NT, NG, 2], bf16, isOutput=False)
    wt = nc.declare_dram_parameter("wt", [P, NT, CB, Cout], bf16, isOutput=False)
    i2m = nc.declare_dram_parameter("i2m", [P, 64], bf16, isOutput=False)
    out = nc.declare_dram_parameter("out", [Cout, HW], bf16, isOutput=True)

    with tile.TileContext(nc) as tc:
        with tc.tile_pool(name="persist", bufs=1) as pp:
            # tap-0 slices arrive via small SP DMAs so the first gather isn't
            # stuck behind bulk input loads; the k>=1 remainders are loaded on
            # the Pool queue, program-ordered between the first gathers.
            wtb0 = pp.tile([P, 1, CB, Cout], bf16, name="wtb0")
            wtbr = pp.tile([P, NT - 1, CB, Cout], bf16, name="wtbr")
            w2b0 = pp.tile([P, 1, NG, 2], bf16, name="w2b0")
            w2br = pp.tile([P, NT - 1, NG, 2], bf16, name="w2br")
            w2t0 = pp.tile([P, 1, NG, 2], f32, name="w2t0")
            w2tr = pp.tile([P, NT - 1, NG, 2], f32, name="w2tr")
            idxt = pp.tile([P, NT, 2 * HW // 16], i16, name="idxt")
            i2t = pp.tile([P, 64], bf16, name="i2t")

            nc.sync.dma_start(out=i2t[:], in_=i2m[:])
            nc.sync.dma_start(out=idxt[:], in_=idx[:])
            nc.sync.dma_start(out=w2b0[:], in_=w2[:, 0:1])
            nc.sync.dma_start(out=wtb0[:], in_=wt[:, 0:1])
            # upconvert weights on DVE (idle at startup)
            nc.vector.tensor_copy(out=w2t0[:], in_=w2b0[:])

            def w2v(k, g, yh):
                if k == 0:
                    return w2t0[:, 0, g, yh:yh + 1]
                return w2tr[:, k - 1, g, yh:yh + 1]

            def wtv(k, cb, mb):
                if k == 0:
                    return wtb0[:, 0, cb, mb * P:(mb + 1) * P]
                return wtbr[:, k - 1, cb, mb * P:(mb + 1) * P]

            tbl_rows = bass.AP(tbl[:].tensor, 0, [[2 * C, TROWS], [1, 2 * C]])

            HGC = GC // 2            # groups per output-column half (8)
            with (
                tc.tile_pool(name="gather", bufs=4) as g_pool,
                tc.tile_pool(name="prod", bufs=96) as pr_pool,
                tc.tile_pool(name="vout", bufs=8) as v_pool,
                tc.tile_pool(name="obuf", bufs=2) as o_pool,
                tc.tile_pool(name="psum_out", bufs=1, space="PSUM") as pso_pool,
                tc.tile_pool(name="psum_val", bufs=4, space="PSUM") as psv_pool,
            ):
                def do_block(out_ps, ch, k, g0, ng):
                    """One tap's gather+compute for groups [g0, g0+ng)."""
                    nrows = ng * P           # 2 rows per sample, 64/group
                    g = g_pool.tile([P, ng, 2 * C], fp8, name="g")
                    col0 = (ch * GC + g0) * 8
                    nc.gpsimd.dma_gather(
                        g[:], tbl_rows,
                        idxt[:, k, col0:col0 + ng * 8],
                        nrows, nrows, 2 * C, elem_step=2 * C,
                    )
                    # per-group 2-diagonal weight tiles (I2 * w), DVE 4x mode
                    dgs = {}
                    for i in range(ng):
                        gg = g0 + i
                        g_glob = ch * GC + gg
                        dg = pr_pool.tile([P, 2, 64], bf16, name="dg")
                        dgs[gg] = dg
                        for yh in range(2):
                            nc.vector.tensor_scalar(
                                out=dg[:, yh, :], in0=i2t[:],
                                scalar1=w2v(k, g_glob, yh),
                                scalar2=None, op0=Alu.mult,
                            )
                    # val split per (cb, column-half): Act sub-copies start
                    # mid-iteration, so no GEMM waits on a copy in flight
                    vhalf = {}
                    for cb in range(CB):
                        for h0 in range(g0, g0 + ng, HGC):
                            nh = min(HGC, g0 + ng - h0)
                            val_ps = psv_pool.tile([P, nh * 64], f32,
                                                   space="PSUM", name="val_ps")
                            for j2 in range(nh):
                                gg = h0 + j2
                                i = gg - g0
                                for yh in range(2):
                                    nc.tensor.matmul(
                                        out=val_ps[:, j2 * 64:(j2 + 1) * 64],
                                        lhsT=g[:, i,
                                               yh * C + cb * P:
                                               yh * C + (cb + 1) * P],
                                        rhs=dgs[gg][:, yh, :],
                                        start=(yh == 0), stop=(yh == 1),
                                    )
                            vs = v_pool.tile([P, nh * 64], bf16, name="vs")
                            nc.scalar.copy(out=vs[:], in_=val_ps[:])
                            vhalf[(cb, h0)] = (vs, nh)
                    return (out_ps, k, vhalf, g0, ng)

                def emit_gemms(blk):
                    # cb-major order: each GEMM's vs copy completed earliest
                    out_ps, k, vhalf, g0, ng = blk
                    for cb in range(CB):
                        for h0 in range(g0, g0 + ng, HGC):
                            for mb in range(MB):
                                vs, nh = vhalf[(cb, h0)]
                                nsl = slice(h0 * 64, (h0 + nh) * 64)
                                nc.tensor.matmul(
                                    out=out_ps[mb][:, nsl],
                                    lhsT=wtv(k, cb, mb),
                                    rhs=vs[:],
                                    start=(k == 0 and cb == 0),
                                    stop=(k == NT - 1 and cb == CB - 1),
                                )

                def drain_chunk(ch, out_ps):
                    for mb in range(MB):
                        ob = o_pool.tile([P, JC], bf16, name="ob")
                        nc.scalar.copy(out=ob[:], in_=out_ps[mb][:])
                        nc.sync.dma_start(
                            out=out[mb * P:(mb + 1) * P, ch * JC:(ch + 1) * JC],
                            in_=ob[:],
                        )

                # GEMMs of each block are deferred until after the NEXT
                # block's transposes: the Act vs-copies get a whole block of
                # slack, so the PE stream never waits on them
                pending = None   # (blk, drain_ch_or_None)
                for ch in range(n_chunks):
                    out_ps = [
                        pso_pool.tile([P, JC], f32, space="PSUM", name=f"out_ps{m}")
                        for m in range(MB)
                    ]
                    first = ch == 0
                    last_ch = ch == n_chunks - 1
                    for k in range(NT):
                        if (last_ch and k == NT - 1) or (first and k == 0):
                            # split the first/final tap into half-gathers:
                            # shallower pipeline fill and drain
                            sub = [(0, HGC), (HGC, HGC)]
                        else:
                            sub = [(0, GC)]
                        for si, (g0, ng) in enumerate(sub):
                            blk = do_block(out_ps, ch, k, g0, ng)
                            if pending is not None:
                                emit_gemms(pending[0])
                                if pending[1] is not None:
                                    drain_chunk(*pending[1])
                            is_last = (k == NT - 1 and si == len(sub) - 1)
                            pending = (blk, (ch, out_ps) if is_last else None)
                        if first and k == 0:
                            # bulk weight loads ride the Pool queue here so
                            # they cannot front-run the first gathers on the
                            # shared DMA device
                            nc.gpsimd.dma_start(out=w2br[:], in_=w2[:, 1:])
                            nc.gpsimd.dma_start(out=wtbr[:], in_=wt[:, 1:])
                            nc.vector.tensor_copy(out=w2tr[:], in_=w2br[:])
                if pending is not None:
                    emit_gemms(pending[0])
                    if pending[1] is not None:
                        drain_chunk(*pending[1])

    nc.compile()
    return nc


def host_prep(x_b, offset_b, weight, H, W, KH, KW, PAD):
    """Per-core input map from one batch slice (numpy, f32)."""
    C = x_b.shape[0]
    Cout = weight.shape[0]
    HW = H * W
    NT = KH * KW
    CB = C // P
    NG = HW // 64

    # pair-row gather table: entry r=(y0+1)*W+x holds rows (y0, y0+1) at col x
    xt = x_b.reshape(C, H, W).transpose(1, 2, 0).astype(np.float32)  # [H, W, C]
    Z = np.zeros((H + 2, W, C), np.float32)
    Z[1:H + 1] = xt
    T = np.concatenate([Z[0:H + 1], Z[1:H + 2]], axis=-1)  # [(H+1), W, 2C]
    tbl = T.reshape((H + 1) * W, 2 * C).astype(FP8)

    # sample coords per (tap, pixel)
    off = offset_b.reshape(NT, 2, HW).astype(np.float32)
    j = np.arange(HW)
    ks = np.arange(NT)
    by = (j[None, :] // W - PAD + (ks // KW)[:, None]).astype(np.float32)
    bx = (j[None, :] % W - PAD + (ks % KW)[:, None]).astype(np.float32)
    py = by + off[:, 0]
    px = bx + off[:, 1]
    y0 = np.floor(py)
    x0 = np.floor(px)
    ly = (py - y0).astype(np.float32)
    lx = (px - x0).astype(np.float32)
    qy = np.clip(y0, -1, H - 1)
    sx = np.clip(x0, 0, W - 2)
    base = ((qy + 1) * W + sx).astype(np.int32)  # [NT, HW]

    wy0 = (1.0 - ly) * ((y0 >= 0) & (y0 <= H - 1))
    wyB = ly * ((y0 >= -1) & (y0 <= H - 2))
    vx0 = (x0 >= 0) & (x0 <= W - 1)
    vx1 = (x0 >= -1) & (x0 <= W - 2)
    wxA = (1.0 - lx) * vx0 * (x0 == sx) + lx * vx1 * ((x0 + 1) == sx)
    wxB = (1.0 - lx) * vx0 * (x0 == (sx + 1)) + lx * vx1 * ((x0 + 1) == (sx + 1))
    # piece order: [y0|x0, y1|x0, y0|x1, y1|x1]
    w4 = np.stack([wy0 * wxA, wyB * wxA, wy0 * wxB, wyB * wxB],
                  axis=-1).astype(np.float32)  # [NT, HW, 4]

    # two-diagonal weights: partition p of group g holds sample 64g + p//2,
    # x-piece p%2; y-half yh selects row y0/y1 -> piece index yh + 2*(p%2)
    w4r = w4.reshape(NT, NG, 64, 4)
    pp_ = np.arange(P)
    pc = pp_ // 2
    pxp = pp_ % 2
    w2d = np.empty((P, NT, NG, 2), np.float32)
    for yh in range(2):
        piece = yh + 2 * pxp                        # [128]
        sel = w4r[:, :, pc, :]                      # [NT, NG, 128, 4]
        w2d[:, :, :, yh] = sel[:, :, np.arange(P), piece].transpose(2, 0, 1)
    w2d = np.ascontiguousarray(w2d).astype(BF16)

    # interleaved idx rows: sample i -> entries (base, base+1), 16-wrap
    idx2 = np.empty((NT, 2 * HW), np.int32)
    idx2[:, 0::2] = base
    idx2[:, 1::2] = base + 1
    idxw = idx2.reshape(NT, (2 * HW) // 16, 16).transpose(2, 0, 1)
    idxw = np.ascontiguousarray(np.tile(idxw, (8, 1, 1))).astype(np.int16)

    i2m = np.zeros((P, 64), np.float32)
    i2m[pp_, pc] = 1.0
    i2m = i2m.astype(BF16)

    wtd = np.ascontiguousarray(
        weight.reshape(Cout, CB, P, NT).transpose(2, 3, 1, 0)).astype(BF16)
    return {"tbl": tbl, "idx": idxw, "w2": w2d, "wt": wtd, "i2m": i2m}


_NC_CACHE = {}


def _get_nc(key, **kw):
    if key not in _NC_CACHE:
        _NC_CACHE[key] = build_dcn(**kw)
    return _NC_CACHE[key]


def kernel(x, offset, weight):
    x = np.asarray(x, dtype=np.float32)
    offset = np.asarray(offset, dtype=np.float32)
    weight = np.asarray(weight, dtype=np.float32)
    B, C, H, W = x.shape
    Cout = weight.shape[0]
    KH, KW = weight.shape[2], weight.shape[3]
    assert B == 8 and C % 128 == 0 and Cout % 128 == 0
    nc = _get_nc((C, Cout, H, W, KH, KW), C=C, Cout=Cout, H=H, W=W,
                 KH=KH, KW=KW, CHUNK_JT=8)
    in_maps = [host_prep(x[b], offset[b], weight, H, W, KH, KW, 1)
               for b in range(B)]
    res = run_bass_kernel_spmd(nc, in_maps, list(range(B)))
    out = np.stack([
        np.asarray(res.results[b]["out"]).astype(np.float32).reshape(Cout, H, W)
        for b in range(B)
    ])
    return out


# revision 45
# speedup vs baseline: 1.2665x; 1.0010x over previous
"""Self-contained Trainium2 (Bass/Tile) DeformConv2d kernel.

kernel(x, offset, weight) -> np.ndarray [B, Cout, H, W] float32.
Data-parallel over batch: one SPMD Bass program per NeuronCore (8 cores).

Design (v4):
- All bilinear weights / gather indices / gather table are built on the host
  (numpy) and passed as DRAM inputs; no on-device prep phases.
- Gather table is a float8 e3m4 "pair-row" layout: entry (y, x) holds image
  rows y and y+1 at column x concatenated (2C values, 512B). Each sample
  issues TWO 512B descriptors (columns x0, x0+1) landing on adjacent SBUF
  partitions: 64 samples x 2 x-pieces per 128-partition group. e3m4 halves
  gather DMA vs bf16 at ~1.3% output rel err (e4m3 would be ~2.5%, failing
  the 2% gate).
- Bilinear weights enter as 128x64 two-diagonal matrices (I2 mask * per-
  partition weight, built by DVE tensor_scalar in 4x mode; they depend only
  on host data so they run ahead). One PE matmul per (y-half, group, cb)
  multiplies, transposes to channel-major AND reduces both x-pieces:
  out[c, j] = w_x0*g_x0[c, j] + w_x1*g_x1[c, j]; the two y-halves accumulate
  in PSUM. This is 2x fewer PE transpose cycles than one-piece-per-matmul.
- Per-tap GEMM accumulates out[o, j] over (tap, cb) in PSUM; val and out
  PSUM tiles are split per (cb, column-half) so the Activation-engine
  PSUM->SBUF copies pipeline under the PE stream.
- Tap-0 weight slices load via small SP DMAs, the bulk via the Pool queue,
  so the first gathers are not stuck behind input loads; first/last taps
  are half-gathers to shorten pipeline fill/drain.
- Output is written bf16 and upcast on the host.
"""
import sys
import numpy as np
import ml_dtypes

for _p in ("/opt/trn_rl_repo",):
    if _p not in sys.path:
        sys.path.insert(0, _p)

import concourse.bass as bass
import concourse.mybir as mybir
import concourse.tile as tile
from concourse import bacc
from concourse.bass_utils import run_bass_kernel_spmd

f32 = mybir.dt.float32
bf16 = mybir.dt.bfloat16
fp8 = mybir.dt.float8e3
i16 = mybir.dt.int16
Alu = mybir.AluOpType
P = 128
BF16 = ml_dtypes.bfloat16
FP8 = ml_dtypes.float8_e3m4


def build_dcn(C=256, Cout=256, H=64, W=64, KH=3, KW=3, CHUNK_JT=8):
    HW = H * W
    S = HW // P              # 32 pixel slots of 128
    NT = KH * KW             # 9 taps
    CB = C // P              # 2 input-channel blocks
    MB = Cout // P           # 2 output-channel blocks
    assert S % CHUNK_JT == 0
    n_chunks = S // CHUNK_JT
    JC = CHUNK_JT * P        # 1024 pixels per chunk
    NG = HW // 64            # 64-sample groups over the image
    GC = JC // 64            # groups per chunk (16)
    TROWS = (H + 1) * W      # pair-table rows

    nc = bacc.Bacc("TRN2", target_bir_lowering=False, debug=False)

    tbl = nc.declare_dram_parameter("tbl", [TROWS, 2 * C], fp8, isOutput=False)
    # two idx entries per sample (x0, x0+1), 16-wrap layout
    idx = nc.declare_dram_parameter("idx", [P, NT, 2 * HW // 16], i16,
                                    isOutput=False)
    # per-partition 2-diagonal weights: [p, k, group, y-half]
    w2 = nc.declare_dram_parameter("w2", [P, NT, NG, 2], bf16, isOutput=False)
    wt = nc.declare_dram_parameter("wt", [P, NT, CB, Cout], bf16, isOutput=False)
    i2m = nc.declare_dram_parameter("i2m", [P, 64], bf16, isOutput=False)
    out = nc.declare_dram_parameter("out", [Cout, HW], bf16, isOutput=True)

    with tile.TileContext(nc) as tc:
        with tc.tile_pool(name="persist", bufs=1) as pp:
            # tap-0 slices arrive via small SP DMAs so the first gather isn't
            # stuck behind bulk input loads; the k>=1 remainders are loaded on
            # the Pool queue, program-ordered between the first gathers.
            wtb0 = pp.tile([P, 1, CB, Cout], bf16, name="wtb0")
            wtbr = pp.tile([P, NT - 1, CB, Cout], bf16, name="wtbr")
            w2b0 = pp.tile([P, 1, NG, 2], bf16, name="w2b0")
            w2br = pp.tile([P, NT - 1, NG, 2], bf16, name="w2br")
            w2t0 = pp.tile([P, 1, NG, 2], f32, name="w2t0")
            w2tr = pp.tile([P, NT - 1, NG, 2], f32, name="w2tr")
            idxt = pp.tile([P, NT, 2 * HW // 16], i16, name="idxt")
            i2t = pp.tile([P, 64], bf16, name="i2t")

            nc.sync.dma_start(out=i2t[:], in_=i2m[:])
            nc.sync.dma_start(out=idxt[:], in_=idx[:])
            nc.sync.dma_start(out=w2b0[:], in_=w2[:, 0:1])
            nc.sync.dma_start(out=wtb0[:], in_=wt[:, 0:1])
            # upconvert weights on DVE (idle at startup)
            nc.vector.tensor_copy(out=w2t0[:], in_=w2b0[:])

            def w2v(k, g, yh):
                if k == 0:
                    return w2t0[:, 0, g, yh:yh + 1]
                return w2tr[:, k - 1, g, yh:yh + 1]

            def wtv(k, cb, mb):
                if k == 0:
                    return wtb0[:, 0, cb, mb * P:(mb + 1) * P]
                return wtbr[:, k - 1, cb, mb * P:(mb + 1) * P]

            tbl_rows = bass.AP(tbl[:].tensor, 0, [[2 * C, TROWS], [1, 2 * C]])

            HGC = GC // 2            # groups per output-column half (8)
            with (
                tc.tile_pool(name="gather", bufs=4) as g_pool,
                tc.tile_pool(name="prod", bufs=160) as pr_pool,
                tc.tile_pool(name="vout", bufs=16) as v_pool,
                tc.tile_pool(name="obuf", bufs=2) as o_pool,
                tc.tile_pool(name="psum_out", bufs=1, space="PSUM") as pso_pool,
                tc.tile_pool(name="psum_val", bufs=4, space="PSUM") as psv_pool,
            ):
                def do_block(out_ps, ch, k, g0, ng):
                    """One tap's gather+compute for groups [g0, g0+ng)."""
                    nrows = ng * P           # 2 rows per sample, 64/group
                    g = g_pool.tile([P, ng, 2 * C], fp8, name="g")
                    col0 = (ch * GC + g0) * 8
                    nc.gpsimd.dma_gather(
                        g[:], tbl_rows,
                        idxt[:, k, col0:col0 + ng * 8],
                        nrows, nrows, 2 * C, elem_step=2 * C,
                    )
                    # per-group 2-diagonal weight tiles (I2 * w), DVE 4x mode
                    dgs = {}
                    for i in range(ng):
                        gg = g0 + i
                        g_glob = ch * GC + gg
                        dg = pr_pool.tile([P, 2, 64], bf16, name="dg")
                        dgs[gg] = dg
                        for yh in range(2):
                            nc.vector.tensor_scalar(
                                out=dg[:, yh, :], in0=i2t[:],
                                scalar1=w2v(k, g_glob, yh),
                                scalar2=None, op0=Alu.mult,
                            )
                    # val split per (cb, column-half): Act sub-copies start
                    # mid-iteration, so no GEMM waits on a copy in flight
                    vhalf = {}
                    for cb in range(CB):
                        for h0 in range(g0, g0 + ng, HGC):
                            nh = min(HGC, g0 + ng - h0)
                            val_ps = psv_pool.tile([P, nh * 64], f32,
                                                   space="PSUM", name="val_ps")
                            for j2 in range(nh):
                                gg = h0 + j2
                                i = gg - g0
                                for yh in range(2):
                                    nc.tensor.matmul(
                                        out=val_ps[:, j2 * 64:(j2 + 1) * 64],
                                        lhsT=g[:, i,
                                               yh * C + cb * P:
                                               yh * C + (cb + 1) * P],
                                        rhs=dgs[gg][:, yh, :],
                                        start=(yh == 0), stop=(yh == 1),
                                    )
                            vs = v_pool.tile([P, nh * 64], bf16, name="vs")
                            nc.scalar.copy(out=vs[:], in_=val_ps[:])
                            vhalf[(cb, h0)] = (vs, nh)
                    return (out_ps, k, vhalf, g0, ng)

                def emit_gemms(blk):
                    # cb-major order: each GEMM's vs copy completed earliest
                    out_ps, k, vhalf, g0, ng = blk
                    for cb in range(CB):
                        for h0 in range(g0, g0 + ng, HGC):
                            for mb in range(MB):
                                vs, nh = vhalf[(cb, h0)]
                                nsl = slice(h0 * 64, (h0 + nh) * 64)
                                nc.tensor.matmul(
                                    out=out_ps[mb][:, nsl],
                                    lhsT=wtv(k, cb, mb),
                                    rhs=vs[:],
                                    start=(k == 0 and cb == 0),
                                    stop=(k == NT - 1 and cb == CB - 1),
                                )

                def drain_chunk(ch, out_ps):
                    for mb in range(MB):
                        ob = o_pool.tile([P, JC], bf16, name="ob")
                        nc.scalar.copy(out=ob[:], in_=out_ps[mb][:])
                        nc.sync.dma_start(
                            out=out[mb * P:(mb + 1) * P, ch * JC:(ch + 1) * JC],
                            in_=ob[:],
                        )

                # GEMMs of each block are deferred until after the NEXT
                # block's transposes: the Act vs-copies get a whole block of
                # slack, so the PE stream never waits on them
                pending = None   # (blk, drain_ch_or_None)
                for ch in range(n_chunks):
                    out_ps = [
                        pso_pool.tile([P, JC], f32, space="PSUM", name=f"out_ps{m}")
                        for m in range(MB)
                    ]
                    first = ch == 0
                    last_ch = ch == n_chunks - 1
                    for k in range(NT):
                        if (last_ch and k == NT - 1) or (first and k == 0):
                            # split the first/final tap into half-gathers:
                            # shallower pipeline fill and drain
                            sub = [(0, HGC), (HGC, HGC)]
                        else:
                            sub = [(0, GC)]
                        for si, (g0, ng) in enumerate(sub):
                            blk = do_block(out_ps, ch, k, g0, ng)
                            if pending is not None:
                                emit_gemms(pending[0])
                                if pending[1] is not None:
                                    drain_chunk(*pending[1])
                            is_last = (k == NT - 1 and si == len(sub) - 1)
                            pending = (blk, (ch, out_ps) if is_last else None)
                        if first and k == 0:
                            # bulk weight loads ride the Pool queue here so
                            # they cannot front-run the first gathers on the
                            # shared DMA device
                            nc.gpsimd.dma_start(out=w2br[:], in_=w2[:, 1:])
                            nc.gpsimd.dma_start(out=wtbr[:], in_=wt[:, 1:])
                            nc.vector.tensor_copy(out=w2tr[:], in_=w2br[:])
                if pending is not None:
                    emit_gemms(pending[0])
                    if pending[1] is not None:
                        drain_chunk(*pending[1])

    nc.compile()
    return nc


def host_prep(x_b, offset_b, weight, H, W, KH, KW, PAD):
    """Per-core input map from one batch slice (numpy, f32)."""
    C = x_b.shape[0]
    Cout = weight.shape[0]
    HW = H * W
    NT = KH * KW
    CB = C // P
    NG = HW // 64

    # pair-row gather table: entry r=(y0+1)*W+x holds rows (y0, y0+1) at col x
    xt = x_b.reshape(C, H, W).transpose(1, 2, 0).astype(np.float32)  # [H, W, C]
    Z = np.zeros((H + 2, W, C), np.float32)
    Z[1:H + 1] = xt
    T = np.concatenate([Z[0:H + 1], Z[1:H + 2]], axis=-1)  # [(H+1), W, 2C]
    tbl = T.reshape((H + 1) * W, 2 * C).astype(FP8)

    # sample coords per (tap, pixel)
    off = offset_b.reshape(NT, 2, HW).astype(np.float32)
    j = np.arange(HW)
    ks = np.arange(NT)
    by = (j[None, :] // W - PAD + (ks // KW)[:, None]).astype(np.float32)
    bx = (j[None, :] % W - PAD + (ks % KW)[:, None]).astype(np.float32)
    py = by + off[:, 0]
    px = bx + off[:, 1]
    y0 = np.floor(py)
    x0 = np.floor(px)
    ly = (py - y0).astype(np.float32)
    lx = (px - x0).astype(np.float32)
    qy = np.clip(y0, -1, H - 1)
    sx = np.clip(x0, 0, W - 2)
    base = ((qy + 1) * W + sx).astype(np.int32)  # [NT, HW]

    wy0 = (1.0 - ly) * ((y0 >= 0) & (y0 <= H - 1))
    wyB = ly * ((y0 >= -1) & (y0 <= H - 2))
    vx0 = (x0 >= 0) & (x0 <= W - 1)
    vx1 = (x0 >= -1) & (x0 <= W - 2)
    wxA = (1.0 - lx) * vx0 * (x0 == sx) + lx * vx1 * ((x0 + 1) == sx)
    wxB = (1.0 - lx) * vx0 * (x0 == (sx + 1)) + lx * vx1 * ((x0 + 1) == (sx + 1))
    # piece order: [y0|x0, y1|x0, y0|x1, y1|x1]
    w4 = np.stack([wy0 * wxA, wyB * wxA, wy0 * wxB, wyB * wxB],
                  axis=-1).astype(np.float32)  # [NT, HW, 4]

    # two-diagonal weights: partition p of group g holds sample 64g + p//2,
    # x-piece p%2; y-half yh selects row y0/y1 -> piece index yh + 2*(p%2)
    w4r = w4.reshape(NT, NG, 64, 4)
    pp_ = np.arange(P)
    pc = pp_ // 2
    pxp = pp_ % 2
    w2d = np.empty((P, NT, NG, 2), np.float32)
    for yh in range(2):
        piece = yh + 2 * pxp                        # [128]
        sel = w4r[:, :, pc, :]                      # [NT, NG, 128, 4]
        w2d[:, :, :, yh] = sel[:, :, np.arange(P), piece].transpose(2, 0, 1)
    w2d = np.ascontiguousarray(w2d).astype(BF16)

    # interleaved idx rows: sample i -> entries (base, base+1), 16-wrap
    idx2 = np.empty((NT, 2 * HW), np.int32)
    idx2[:, 0::2] = base
    idx2[:, 1::2] = base + 1
    idxw = idx2.reshape(NT, (2 * HW) // 16, 16).transpose(2, 0, 1)
    idxw = np.ascontiguousarray(np.tile(idxw, (8, 1, 1))).astype(np.int16)

    i2m = np.zeros((P, 64), np.float32)
    i2m[pp_, pc] = 1.0
    i2m = i2m.astype(BF16)

    wtd = np.ascontiguousarray(
        weight.reshape(Cout, CB, P, NT).transpose(2, 3, 1, 0)).astype(BF16)
    return {"tbl": tbl, "idx": idxw, "w2": w2d, "wt": wtd, "i2m": i2m}


_NC_CACHE = {}


def _get_nc(key, **kw):
    if key not in _NC_CACHE:
        _NC_CACHE[key] = build_dcn(**kw)
    return _NC_CACHE[key]


def kernel(x, offset, weight):
    x = np.asarray(x, dtype=np.float32)
    offset = np.asarray(offset, dtype=np.float32)
    weight = np.asarray(weight, dtype=np.float32)
    B, C, H, W = x.shape
    Cout = weight.shape[0]
    KH, KW = weight.shape[2], weight.shape[3]
    assert B == 8 and C % 128 == 0 and Cout % 128 == 0
    nc = _get_nc((C, Cout, H, W, KH, KW), C=C, Cout=Cout, H=H, W=W,
                 KH=KH, KW=KW, CHUNK_JT=8)
    in_maps = [host_prep(x[b], offset[b], weight, H, W, KH, KW, 1)
               for b in range(B)]
    res = run_bass_kernel_spmd(nc, in_maps, list(range(B)))
    out = np.stack([
        np.asarray(res.results[b]["out"]).astype(np.float32).reshape(Cout, H, W)
        for b in range(B)
    ])
    return out
